# revision 6
# baseline (speedup 1.0000x reference)
"""Trainium2 Bass kernel for nn_DualAxisAggAttn (dual-axis aggregation attention).

Reference semantics per batch image x[C=256, H=64, W=64], twice (W axis then H axis):
  qkv = conv1x1(x) -> {q:[1], k:[C], v:[C]};  s = softmax_axis(q)
  ctx[c,a] = sum_r k*s;  out = x + sigmoid(v) * ctx_bcast;  y = conv1x1(out)

Distribution: data-parallel over batch (16 images -> 2 per NeuronCore x 8 cores).

v2 structural optimizations over the phase-interleaved baseline:
  - STAGE COLLAPSE: every stage-H op on x_w = WfW @ out_W is linear in
    channels, so WfW is folded into stage-H's weights on the HOST
    (qH' = qH@WfW, WvH' = WvH@WfW, WkH' = WkH@WfW, F2 = WfH@WfW). The
    stage-W fusion conv never runs on device; out_W is materialized once
    (O = x + g2) and feeds all H matmuls. Removes 2 of 8 big matmul
    passes AND the stage-W PSUM->SBUF eviction on the ACT engine.
  - gate via AF.Sigmoid directly (table set {exp, sigmoid, copy} still
    fits): kills the tensor_scalar "+1" pass and the 0.5-folding.
  - key-path linearity kept: ctx = Wk @ (sum_r x*E) / S.
  - q row replicated 128x in its m-tile -> exp(q) lands partition-broadcast.
  - all matmuls bf16; reductions via contiguous binary trees, u-trees
    merged across the two c-tile groups.
"""

import numpy as np
import ml_dtypes
from contextlib import ExitStack

import concourse.bass as bass
import concourse.bacc as bacc
import concourse.tile as tile
import concourse.mybir as mybir
from concourse.bass_utils import run_bass_kernel_spmd

F32 = mybir.dt.float32
BF16 = mybir.dt.bfloat16
AF = mybir.ActivationFunctionType
ALU = mybir.AluOpType
AX = mybir.AxisListType
NPBF = ml_dtypes.bfloat16

B, C, H, W = 16, 256, 64, 64
HW = H * W
NCORES = 8
BPC = B // NCORES
KT = 2
CH = 512
NCH = HW // CH
GRP = CH // 64

# experiment toggles (part of the build cache key)
O_ON_GPSIMD = False

_BUILD_CACHE = {}
LAST_RESULTS = None


class _Stage:
    """One attention stage for one batch: p1 (qkv+u), p2 (reduce+ctx), p3."""

    def __init__(self, nc, pools, axis_w, src, stat, wk, bias):
        self.nc, self.axis_w = nc, axis_w
        self.src, self.stat, self.wk, self.bias = src, stat, wk, bias
        (self.pbig, self.pgate, self.pchunk, self.pctx, self.pq, self.pv,
         self.pf, self.phv) = pools

    def p1_alloc(self):
        self.E = self.pbig.tile([128, HW], BF16, tag="E")
        self.gate = self.pgate.tile([128, 2, HW], BF16, tag="gate")
        self.u = self.pbig.tile([128, 2, HW], BF16, tag="u")

    def p1_chunk(self, j):
        nc, src, stat, bias = self.nc, self.src, self.stat, self.bias
        sl = bass.ts(j, CH)
        bv2 = bias.get("bv2")
        ps_q = self.pq.tile([128, CH], F32, tag="q")
        ps_v = self.pv.tile([128, 2 * CH], F32, tag="vf")
        for kt in range(KT):
            st, sp = kt == 0, kt == KT - 1
            rhs = src[:, kt, sl]
            nc.tensor.matmul(ps_q[:], stat[:, kt, 2, :], rhs, start=st, stop=sp)
            nc.tensor.matmul(ps_v[:, 0:CH], stat[:, kt, 0, :], rhs, start=st, stop=sp)
            nc.tensor.matmul(ps_v[:, CH:], stat[:, kt, 1, :], rhs, start=st, stop=sp)
        nc.scalar.activation(self.E[:, sl], ps_q[:], AF.Exp, bias=bias["zb"])
        if bv2 is None:
            nc.scalar.activation(
                self.gate[:, :, sl], ps_v[:].rearrange("p (c n) -> p c n", c=2),
                AF.Sigmoid,
            )
        else:
            nc.scalar.activation(self.gate[:, 0, sl], ps_v[:, 0:CH], AF.Sigmoid, bias=bv2[0])
            nc.scalar.activation(self.gate[:, 1, sl], ps_v[:, CH:], AF.Sigmoid, bias=bv2[1])
        eb = self.E[:, sl].unsqueeze(1).broadcast_to([128, 2, CH])
        nc.vector.tensor_tensor(self.u[:, :, sl], src[:, :, sl], eb, op=ALU.mult)

    def p2(self):
        nc, pctx, phv = self.nc, self.pctx, self.phv
        eh_t = phv.tile([128, 2048], BF16, tag="ehv")
        uh_t = phv.tile([128, 4096], BF16, tag="uhv")
        S = pctx.tile([128, 64], F32, tag="S")
        xep = pctx.tile([128, 2, 64], F32, tag="xep")
        if self.axis_w:
            # reduce over w (inner 64): halve to 32, halve in place to 16, TR
            e3 = self.E[:].rearrange("p (a r) -> p a r", r=64)
            ehv = eh_t[:].rearrange("p (a r) -> p a r", r=32)
            nc.vector.tensor_tensor(ehv[:, :, :], e3[:, :, 0:32], e3[:, :, 32:64], op=ALU.add)
            nc.vector.tensor_tensor(ehv[:, :, 0:16], ehv[:, :, 0:16], ehv[:, :, 16:32], op=ALU.add)
            nc.vector.tensor_reduce(S[:], ehv[:, :, 0:16], axis=AX.X, op=ALU.add)
            u4 = self.u[:].rearrange("p c (a r) -> p c a r", r=64)
            uhv = uh_t[:].rearrange("p (c a r) -> p c a r", c=2, r=32)
            nc.vector.tensor_tensor(uhv[:, :, :, :], u4[:, :, :, 0:32], u4[:, :, :, 32:64], op=ALU.add)
            nc.vector.tensor_tensor(uhv[:, :, :, 0:16], uhv[:, :, :, 0:16], uhv[:, :, :, 16:32], op=ALU.add)
            nc.vector.tensor_reduce(xep[:], uhv[:, :, :, 0:16], axis=AX.X, op=ALU.add)
        else:
            # reduce over h (outer): contiguous halving trees
            et = eh_t[:]
            nc.vector.tensor_tensor(et[:, 0:2048], self.E[:, 0:2048], self.E[:, 2048:4096], op=ALU.add)
            n = 1024
            while n >= 128:
                nc.vector.tensor_tensor(et[:, 0:n], et[:, 0:n], et[:, n:2 * n], op=ALU.add)
                n //= 2
            nc.vector.tensor_tensor(S[:], et[:, 0:64], et[:, 64:128], op=ALU.add)
            ut = uh_t[:].rearrange("p (c n) -> p c n", c=2)
            nc.vector.tensor_tensor(ut[:, :, 0:2048], self.u[:, :, 0:2048], self.u[:, :, 2048:4096], op=ALU.add)
            n = 1024
            while n >= 128:
                nc.vector.tensor_tensor(ut[:, :, 0:n], ut[:, :, 0:n], ut[:, :, n:2 * n], op=ALU.add)
                n //= 2
            nc.vector.tensor_tensor(xep[:], ut[:, :, 0:64], ut[:, :, 64:128], op=ALU.add)

        R = pctx.tile([128, 64], F32, tag="R")
        nc.vector.reciprocal(R[:], S[:])
        xn = pctx.tile([128, 2, 64], BF16, tag="xn")
        rb = R[:].unsqueeze(1).broadcast_to([128, 2, 64])
        nc.vector.tensor_tensor(xn[:], xep[:], rb, op=ALU.mult)

        bk2 = self.bias.get("bk2")
        ctx_t = pctx.tile([128, 2, 64], BF16, tag="ctx")
        for mt in range(2):
            ps_c = self.pq.tile([128, 64], F32, tag="q")
            for ct in range(2):
                nc.tensor.matmul(ps_c[:], self.wk[:, ct, mt, :], xn[:, ct, :],
                                 start=ct == 0, stop=ct == 1)
            if bk2 is None:
                nc.vector.tensor_copy(ctx_t[:, mt, :], ps_c[:])
            else:
                nc.vector.tensor_scalar(ctx_t[:, mt, :], ps_c[:], 1.0, bk2[mt],
                                        op0=ALU.mult, op1=ALU.add)
        self.ctx_t = ctx_t

    def g2_chunk(self, j):
        """g2 = gate * ctx_broadcast for chunk j -> [128, 2, GRP, 64] tile."""
        nc = self.nc
        g2 = self.pchunk.tile([128, 2, GRP, 64], BF16, tag="g2")
        gv = self.gate[:, :, bass.ts(j, CH)].rearrange("p c (a r) -> p c a r", r=64)
        if self.axis_w:
            cb = self.ctx_t[:, :, bass.ts(j, GRP)].unsqueeze(3).broadcast_to([128, 2, GRP, 64])
        else:
            cb = self.ctx_t[:].unsqueeze(2).broadcast_to([128, 2, GRP, 64])
        nc.vector.tensor_tensor(g2[:], gv, cb, op=ALU.mult)
        return g2


def _build(flags):
    bvW0, bkW0, bvH0, bkH0, byH0 = flags
    nc = bacc.Bacc(trn_type="TRN2", target_bir_lowering=False, debug=False)

    x_d = nc.dram_tensor("x", [BPC, C, HW], BF16, kind="ExternalInput").ap()
    statW_d = nc.dram_tensor("statW", [128, KT, 3, 128], BF16, kind="ExternalInput").ap()
    statH_d = nc.dram_tensor("statH", [128, KT, 3, 128], BF16, kind="ExternalInput").ap()
    wkW_d = nc.dram_tensor("wkW", [128, KT, 2, 128], BF16, kind="ExternalInput").ap()
    wkH_d = nc.dram_tensor("wkH", [128, KT, 2, 128], BF16, kind="ExternalInput").ap()
    f2_d = nc.dram_tensor("f2", [128, KT, 2, 128], BF16, kind="ExternalInput").ap()
    fg_d = nc.dram_tensor("fg", [128, KT, 2, 128], BF16, kind="ExternalInput").ap()
    bias_d = nc.dram_tensor("biases", [5, 2, 128], F32, kind="ExternalInput").ap()
    y_d = nc.dram_tensor("y", [BPC, C, HW], F32, kind="ExternalOutput").ap()

    with tile.TileContext(nc) as tc, ExitStack() as ctx:
        wp = ctx.enter_context(tc.tile_pool(name="weights", bufs=1))
        xp = ctx.enter_context(tc.tile_pool(name="x", bufs=2))
        op_ = ctx.enter_context(tc.tile_pool(name="O", bufs=2))
        pbig = ctx.enter_context(tc.tile_pool(name="big", bufs=2))
        pgate = ctx.enter_context(tc.tile_pool(name="gate", bufs=3))
        pchunk = ctx.enter_context(tc.tile_pool(name="chunk", bufs=3))
        pctx = ctx.enter_context(tc.tile_pool(name="ctx", bufs=3))
        phv = ctx.enter_context(tc.tile_pool(name="hv", bufs=1))
        yp = ctx.enter_context(tc.tile_pool(name="yev", bufs=3))
        pq = ctx.enter_context(tc.tile_pool(name="psq", bufs=2, space="PSUM"))
        pvf = ctx.enter_context(tc.tile_pool(name="psvf", bufs=3, space="PSUM"))
        pools = (pbig, pgate, pchunk, pctx, pq, pvf, pvf, phv)

        def wload(name, dram, shape, dt):
            t = wp.tile(shape, dt, tag=name)
            nc.scalar.dma_start(t[:], dram[:])
            return t

        statW = wload("statW", statW_d, [128, KT, 3, 128], BF16)
        statH = wload("statH", statH_d, [128, KT, 3, 128], BF16)
        wkW = wload("wkW", wkW_d, [128, KT, 2, 128], BF16)
        wkH = wload("wkH", wkH_d, [128, KT, 2, 128], BF16)
        f2 = wload("f2", f2_d, [128, KT, 2, 128], BF16)
        fg = wload("fg", fg_d, [128, KT, 2, 128], BF16)

        bias_sb = wp.tile([128, 5, 2], F32, tag="biases")
        nc.scalar.dma_start(bias_sb[:], bias_d[:].transpose([2, 0, 1]))
        zb = wp.tile([128, 1], F32, tag="zb")
        nc.vector.memset(zb[:], 0.0)

        def bap(i, ct):
            return bias_sb[:, i, ct].unsqueeze(1)

        biasW = {
            "bv2": None if bvW0 else [bap(0, ct) for ct in range(2)],
            "bk2": None if bkW0 else [bap(1, ct) for ct in range(2)],
            "zb": zb[:],
        }
        biasH = {
            "bv2": None if bvH0 else [bap(2, ct) for ct in range(2)],
            "bk2": None if bkH0 else [bap(3, ct) for ct in range(2)],
            "zb": zb[:],
        }

        def load_x(b):
            x = xp.tile([128, KT, HW], BF16, tag="x")
            for kt in range(KT):
                for j in range(NCH):
                    nc.sync.dma_start(x[:, kt, bass.ts(j, CH)],
                                      x_d[b, bass.ts(kt, 128), bass.ts(j, CH)])
            return x

        def make_O(b):
            O_t = op_.tile([128, KT, HW], BF16, tag="O")
            return O_t

        # stage-W p3 chunk: O = x + gate*ctxb  (no matmul; feeds all H work)
        def w_p3_chunk(st, x, O, j):
            g2 = st.g2_chunk(j)
            g2f = g2[:].rearrange("p c a r -> p c (a r)")
            eng = nc.gpsimd if O_ON_GPSIMD else nc.vector
            eng.tensor_tensor(O[:, :, bass.ts(j, CH)], x[:, :, bass.ts(j, CH)], g2f, op=ALU.add)

        # stage-H p3 chunk: y = F2@O + FG@g2H, evict
        def h_p3_chunk(st, O, b, j):
            g2 = st.g2_chunk(j)
            ps_f = pvf.tile([128, 2 * CH], F32, tag="vf")
            sl = bass.ts(j, CH)
            for mt in range(2):
                half = ps_f[:, bass.ts(mt, CH)]
                nc.tensor.matmul(half, f2[:, 0, mt, :], O[:, 0, sl], start=True, stop=False)
                nc.tensor.matmul(half, f2[:, 1, mt, :], O[:, 1, sl], start=False, stop=False)
                nc.tensor.matmul(half, fg[:, 0, mt, :], g2[:, 0].rearrange("p a r -> p (a r)"), start=False, stop=False)
                nc.tensor.matmul(half, fg[:, 1, mt, :], g2[:, 1].rearrange("p a r -> p (a r)"), start=False, stop=True)
            y_t = yp.tile([128, 2, CH], F32, tag="y")
            if byH0:
                nc.scalar.activation(y_t[:], ps_f[:].rearrange("p (c n) -> p c n", c=2), AF.Copy)
            else:
                for ct in range(2):
                    nc.scalar.activation(y_t[:, ct, :], ps_f[:, bass.ts(ct, CH)],
                                         AF.Identity, bias=bap(4, ct))
            nc.sync.dma_start(
                y_d[b].rearrange("(c p) n -> p c n", p=128)[:, :, sl], y_t[:])

        x0 = load_x(0)
        x1 = load_x(1)
        w0 = _Stage(nc, pools, True, x0[:], statW, wkW, biasW)
        w1 = _Stage(nc, pools, True, x1[:], statW, wkW, biasW)

        def run_p1(st):
            st.p1_alloc()
            for j in range(NCH):
                st.p1_chunk(j)

        run_p1(w0)
        run_p1(w1)
        w0.p2()

        O0 = make_O(0)
        h0 = _Stage(nc, pools, False, O0[:], statH, wkH, biasH)
        h0.p1_alloc()
        for j in range(NCH):
            w_p3_chunk(w0, x0, O0, j)
            h0.p1_chunk(j)

        w1.p2()
        O1 = make_O(1)
        h1 = _Stage(nc, pools, False, O1[:], statH, wkH, biasH)
        h1.p1_alloc()
        for j in range(NCH):
            w_p3_chunk(w1, x1, O1, j)
            h1.p1_chunk(j)

        h0.p2()
        for j in range(NCH):
            h_p3_chunk(h0, O0, 0, j)
        h1.p2()
        for j in range(NCH):
            h_p3_chunk(h1, O1, 1, j)

    nc.compile()
    return nc


def _to_stat(wq, wv):
    """[q replicated; v] -> lhsT layout [128, KT, 3, 128]."""
    stat = np.empty((128, KT, 3, 128), np.float32)
    for kt in range(KT):
        cs = slice(kt * 128, (kt + 1) * 128)
        stat[:, kt, 0, :] = wv[0:128, cs].T
        stat[:, kt, 1, :] = wv[128:256, cs].T
        stat[:, kt, 2, :] = np.repeat(wq[cs][:, None], 128, axis=1)
    return stat


def _to_lhsT(w):
    """[256out, 256in] -> [128, KT, 2, 128] (k-tile, m-tile)."""
    t = np.empty((128, KT, 2, 128), np.float32)
    for kt in range(KT):
        cs = slice(kt * 128, (kt + 1) * 128)
        t[:, kt, 0, :] = w[0:128, cs].T
        t[:, kt, 1, :] = w[128:256, cs].T
    return t


def kernel(x, qkvW_w, qkvW_b, qkvH_w, qkvH_b, fusW_w, fusW_b, fusH_w, fusH_b):
    global LAST_RESULTS
    x = np.asarray(x, np.float32)
    f64 = lambda a: np.asarray(a, np.float64)
    qkvW_w, qkvW_b = f64(qkvW_w), f64(qkvW_b)
    qkvH_w, qkvH_b = f64(qkvH_w), f64(qkvH_b)
    fusW_w, fusW_b = f64(fusW_w), f64(fusW_b)
    fusH_w, fusH_b = f64(fusH_w), f64(fusH_b)

    wqW, wkW_m, wvW = qkvW_w[0], qkvW_w[1:1 + C], qkvW_w[1 + C:]
    wqH, wkH_m, wvH = qkvH_w[0], qkvH_w[1:1 + C], qkvH_w[1 + C:]

    # collapse WfW into stage-H weights (x_w = WfW @ O + bfW)
    wqHp = wqH @ fusW_w
    wvHp = wvH @ fusW_w
    wkHp = wkH_m @ fusW_w
    F2 = fusH_w @ fusW_w

    statW = _to_stat(wqW.astype(np.float32), wvW.astype(np.float32))
    statH = _to_stat(wqHp.astype(np.float32), wvHp.astype(np.float32))
    wkWl = _to_lhsT(wkW_m.astype(np.float32))
    wkHl = _to_lhsT(wkHp.astype(np.float32))
    f2l = _to_lhsT(F2.astype(np.float32))
    fgl = _to_lhsT(fusH_w.astype(np.float32))

    # bias constants (all zero for the graded inputs; exact host algebra)
    bvW = qkvW_b[1 + C:]
    bkW = qkvW_b[1:1 + C]
    bvHp = wvH @ fusW_b + qkvH_b[1 + C:]
    bkHp = wkH_m @ fusW_b + qkvH_b[1:1 + C]   # sum_h softmax = 1 -> adds to ctx
    byH = fusH_w @ fusW_b + fusH_b
    biases = np.stack([
        bvW.reshape(2, 128), bkW.reshape(2, 128),
        bvHp.reshape(2, 128), bkHp.reshape(2, 128),
        byH.reshape(2, 128),
    ]).astype(np.float32)

    flags = (
        not bvW.any(), not bkW.any(), not bvHp.any(), not bkHp.any(), not byH.any(),
    )
    if flags not in _BUILD_CACHE:
        _BUILD_CACHE[flags] = _build(flags)
    nc = _BUILD_CACHE[flags]

    tobf = lambda a: np.ascontiguousarray(a.astype(NPBF))
    xbf = np.ascontiguousarray(x.reshape(B, C, HW).astype(NPBF))
    in_maps = []
    for core in range(NCORES):
        in_maps.append({
            "x": xbf[core * BPC: (core + 1) * BPC],
            "statW": tobf(statW), "statH": tobf(statH),
            "wkW": tobf(wkWl), "wkH": tobf(wkHl),
            "f2": tobf(f2l), "fg": tobf(fgl),
            "biases": biases,
        })

    res = run_bass_kernel_spmd(nc, in_maps, list(range(NCORES)))
    LAST_RESULTS = res
    y = np.concatenate([r["y"] for r in res.results], axis=0)
    return y.reshape(B, C, H, W)


# revision 10
# speedup vs baseline: 1.2646x; 1.2646x over previous
"""Trainium2 Bass kernel for nn_DualAxisAggAttn (dual-axis aggregation attention).

Reference semantics per batch image x[C=256, H=64, W=64], twice (W axis then H axis):
  qkv = conv1x1(x) -> {q:[1], k:[C], v:[C]};  s = softmax_axis(q)
  ctx[c,a] = sum_r k*s;  out = x + sigmoid(v) * ctx_bcast;  y = conv1x1(out)

Distribution: data-parallel over batch (16 images -> 2 per NeuronCore x 8 cores).

v2 structural optimizations over the phase-interleaved baseline:
  - STAGE COLLAPSE: every stage-H op on x_w = WfW @ out_W is linear in
    channels, so WfW is folded into stage-H's weights on the HOST
    (qH' = qH@WfW, WvH' = WvH@WfW, WkH' = WkH@WfW, F2 = WfH@WfW). The
    stage-W fusion conv never runs on device; out_W is materialized once
    (O = x + g2) and feeds all H matmuls. Removes 2 of 8 big matmul
    passes AND the stage-W PSUM->SBUF eviction on the ACT engine.
  - gate via AF.Sigmoid directly (table set {exp, sigmoid, copy} still
    fits): kills the tensor_scalar "+1" pass and the 0.5-folding.
  - key-path linearity kept: ctx = Wk @ (sum_r x*E) / S.
  - q row replicated 128x in its m-tile -> exp(q) lands partition-broadcast.
  - all matmuls bf16; reductions via contiguous binary trees, u-trees
    merged across the two c-tile groups.
"""

import numpy as np
import ml_dtypes
from contextlib import ExitStack

import concourse.bass as bass
import concourse.bacc as bacc
import concourse.tile as tile
import concourse.mybir as mybir
from concourse.bass_utils import run_bass_kernel_spmd

F32 = mybir.dt.float32
BF16 = mybir.dt.bfloat16
AF = mybir.ActivationFunctionType
ALU = mybir.AluOpType
AX = mybir.AxisListType
NPBF = ml_dtypes.bfloat16

B, C, H, W = 16, 256, 64, 64
HW = H * W
NCORES = 8
BPC = B // NCORES
KT = 2
CH = 512
NCH = HW // CH
GRP = CH // 64

# experiment toggles (part of the build cache key)
O_ON_GPSIMD = False

_BUILD_CACHE = {}
LAST_RESULTS = None


class _Stage:
    """One attention stage for one batch: p1 (qkv+u), p2 (reduce+ctx), p3."""

    def __init__(self, nc, pools, axis_w, src, stat, wk, bias):
        self.nc, self.axis_w = nc, axis_w
        self.src, self.stat, self.wk, self.bias = src, stat, wk, bias
        (self.pbig, self.pgate, self.pchunk, self.pctx, self.pq, self.pv,
         self.pf, self.phv) = pools

    def p1_alloc(self):
        self.E = self.pbig.tile([128, HW], BF16, tag="E")
        self.gate = self.pgate.tile([128, 2, HW], BF16, tag="gate")
        self.u = self.pbig.tile([128, 2, HW], BF16, tag="u")

    def p1_chunk(self, j):
        nc, src, stat, bias = self.nc, self.src, self.stat, self.bias
        sl = bass.ts(j, CH)
        bv2 = bias.get("bv2")
        ps_q = self.pq.tile([128, CH], F32, tag="q")
        ps_v = self.pv.tile([128, 2 * CH], F32, tag="vf")
        for kt in range(KT):
            st, sp = kt == 0, kt == KT - 1
            rhs = src[:, kt, sl]
            nc.tensor.matmul(ps_q[:], stat[:, kt, 2, :], rhs, start=st, stop=sp)
            nc.tensor.matmul(ps_v[:, 0:CH], stat[:, kt, 0, :], rhs, start=st, stop=sp)
            nc.tensor.matmul(ps_v[:, CH:], stat[:, kt, 1, :], rhs, start=st, stop=sp)
        nc.scalar.activation(self.E[:, sl], ps_q[:], AF.Exp, bias=bias["zb"])
        # sigmoid(v) = (1+tanh(v/2))/2: tanh shares the ACT table set with
        # exp/copy (AF.Sigmoid does NOT -- a table swap costs 1.3us each).
        # The +1 lands on DVE as a cheap in-place tensor_scalar; the /2 is
        # folded into the ctx scale in p2.
        if bv2 is None:
            nc.scalar.activation(
                self.gate[:, :, sl], ps_v[:].rearrange("p (c n) -> p c n", c=2),
                AF.Tanh, scale=0.5,
            )
        else:
            nc.scalar.activation(self.gate[:, 0, sl], ps_v[:, 0:CH], AF.Tanh, bias=bv2[0], scale=0.5)
            nc.scalar.activation(self.gate[:, 1, sl], ps_v[:, CH:], AF.Tanh, bias=bv2[1], scale=0.5)
        nc.vector.tensor_scalar_add(self.gate[:, :, sl], self.gate[:, :, sl], 1.0)
        eb = self.E[:, sl].unsqueeze(1).broadcast_to([128, 2, CH])
        nc.vector.tensor_tensor(self.u[:, :, sl], src[:, :, sl], eb, op=ALU.mult)

    def p2(self):
        nc, pctx, phv = self.nc, self.pctx, self.phv
        eh_t = phv.tile([128, 2048], BF16, tag="ehv")
        uh_t = phv.tile([128, 4096], BF16, tag="uhv")
        S = pctx.tile([128, 64], F32, tag="S")
        xep = pctx.tile([128, 2, 64], F32, tag="xep")
        if self.axis_w:
            # reduce over w (inner 64): halve to 32, halve in place to 16, TR
            e3 = self.E[:].rearrange("p (a r) -> p a r", r=64)
            ehv = eh_t[:].rearrange("p (a r) -> p a r", r=32)
            nc.vector.tensor_tensor(ehv[:, :, :], e3[:, :, 0:32], e3[:, :, 32:64], op=ALU.add)
            nc.vector.tensor_tensor(ehv[:, :, 0:16], ehv[:, :, 0:16], ehv[:, :, 16:32], op=ALU.add)
            nc.vector.tensor_tensor(ehv[:, :, 0:8], ehv[:, :, 0:8], ehv[:, :, 8:16], op=ALU.add)
            nc.vector.tensor_reduce(S[:], ehv[:, :, 0:8], axis=AX.X, op=ALU.add)
            u4 = self.u[:].rearrange("p c (a r) -> p c a r", r=64)
            uhv = uh_t[:].rearrange("p (c a r) -> p c a r", c=2, r=32)
            nc.vector.tensor_tensor(uhv[:, :, :, :], u4[:, :, :, 0:32], u4[:, :, :, 32:64], op=ALU.add)
            nc.vector.tensor_tensor(uhv[:, :, :, 0:16], uhv[:, :, :, 0:16], uhv[:, :, :, 16:32], op=ALU.add)
            nc.vector.tensor_tensor(uhv[:, :, :, 0:8], uhv[:, :, :, 0:8], uhv[:, :, :, 8:16], op=ALU.add)
            nc.vector.tensor_reduce(xep[:], uhv[:, :, :, 0:8], axis=AX.X, op=ALU.add)
        else:
            # reduce over h (outer): contiguous halving trees
            et = eh_t[:]
            nc.vector.tensor_tensor(et[:, 0:2048], self.E[:, 0:2048], self.E[:, 2048:4096], op=ALU.add)
            n = 1024
            while n >= 128:
                nc.vector.tensor_tensor(et[:, 0:n], et[:, 0:n], et[:, n:2 * n], op=ALU.add)
                n //= 2
            nc.vector.tensor_tensor(S[:], et[:, 0:64], et[:, 64:128], op=ALU.add)
            ut = uh_t[:].rearrange("p (c n) -> p c n", c=2)
            nc.vector.tensor_tensor(ut[:, :, 0:2048], self.u[:, :, 0:2048], self.u[:, :, 2048:4096], op=ALU.add)
            n = 1024
            while n >= 128:
                nc.vector.tensor_tensor(ut[:, :, 0:n], ut[:, :, 0:n], ut[:, :, n:2 * n], op=ALU.add)
                n //= 2
            nc.vector.tensor_tensor(xep[:], ut[:, :, 0:64], ut[:, :, 64:128], op=ALU.add)

        R = pctx.tile([128, 64], F32, tag="R")
        nc.vector.reciprocal(R[:], S[:])
        xn = pctx.tile([128, 2, 64], BF16, tag="xn")
        rb = R[:].unsqueeze(1).broadcast_to([128, 2, 64])
        nc.vector.tensor_tensor(xn[:], xep[:], rb, op=ALU.mult)

        bk2 = self.bias.get("bk2")
        ctx_t = pctx.tile([128, 2, 64], BF16, tag="ctx")
        for mt in range(2):
            ps_c = self.pq.tile([128, 64], F32, tag="q")
            for ct in range(2):
                nc.tensor.matmul(ps_c[:], self.wk[:, ct, mt, :], xn[:, ct, :],
                                 start=ct == 0, stop=ct == 1)
            if bk2 is None:
                nc.vector.tensor_scalar_mul(ctx_t[:, mt, :], ps_c[:], 0.5)
            else:
                nc.vector.tensor_scalar(ctx_t[:, mt, :], ps_c[:], 0.5, bk2[mt],
                                        op0=ALU.mult, op1=ALU.add)
        self.ctx_t = ctx_t

    def g2_chunk(self, j):
        """g2 = gate * ctx_broadcast for chunk j -> [128, 2, GRP, 64] tile."""
        nc = self.nc
        g2 = self.pchunk.tile([128, 2, GRP, 64], BF16, tag="g2")
        gv = self.gate[:, :, bass.ts(j, CH)].rearrange("p c (a r) -> p c a r", r=64)
        if self.axis_w:
            cb = self.ctx_t[:, :, bass.ts(j, GRP)].unsqueeze(3).broadcast_to([128, 2, GRP, 64])
        else:
            cb = self.ctx_t[:].unsqueeze(2).broadcast_to([128, 2, GRP, 64])
        nc.vector.tensor_tensor(g2[:], gv, cb, op=ALU.mult)
        return g2


def _build(flags):
    bvW0, bkW0, bvH0, bkH0, byH0 = flags
    nc = bacc.Bacc(trn_type="TRN2", target_bir_lowering=False, debug=False)

    x_d = nc.dram_tensor("x", [BPC, C, HW], BF16, kind="ExternalInput").ap()
    statW_d = nc.dram_tensor("statW", [128, KT, 3, 128], BF16, kind="ExternalInput").ap()
    statH_d = nc.dram_tensor("statH", [128, KT, 3, 128], BF16, kind="ExternalInput").ap()
    wkW_d = nc.dram_tensor("wkW", [128, KT, 2, 128], BF16, kind="ExternalInput").ap()
    wkH_d = nc.dram_tensor("wkH", [128, KT, 2, 128], BF16, kind="ExternalInput").ap()
    f2_d = nc.dram_tensor("f2", [128, KT, 2, 128], BF16, kind="ExternalInput").ap()
    fg_d = nc.dram_tensor("fg", [128, KT, 2, 128], BF16, kind="ExternalInput").ap()
    bias_d = nc.dram_tensor("biases", [5, 2, 128], F32, kind="ExternalInput").ap()
    y_d = nc.dram_tensor("y", [BPC, C, HW], F32, kind="ExternalOutput").ap()

    with tile.TileContext(nc) as tc, ExitStack() as ctx:
        wp = ctx.enter_context(tc.tile_pool(name="weights", bufs=1))
        xp = ctx.enter_context(tc.tile_pool(name="x", bufs=2))
        op_ = ctx.enter_context(tc.tile_pool(name="O", bufs=2))
        pbig = ctx.enter_context(tc.tile_pool(name="big", bufs=2))
        pgate = ctx.enter_context(tc.tile_pool(name="gate", bufs=3))
        pchunk = ctx.enter_context(tc.tile_pool(name="chunk", bufs=3))
        pctx = ctx.enter_context(tc.tile_pool(name="ctx", bufs=3))
        phv = ctx.enter_context(tc.tile_pool(name="hv", bufs=1))
        yp = ctx.enter_context(tc.tile_pool(name="yev", bufs=3))
        pq = ctx.enter_context(tc.tile_pool(name="psq", bufs=2, space="PSUM"))
        pvf = ctx.enter_context(tc.tile_pool(name="psvf", bufs=3, space="PSUM"))
        pools = (pbig, pgate, pchunk, pctx, pq, pvf, pvf, phv)

        def wload(name, dram, shape, dt):
            t = wp.tile(shape, dt, tag=name)
            nc.scalar.dma_start(t[:], dram[:])
            return t

        statW = wload("statW", statW_d, [128, KT, 3, 128], BF16)
        statH = wload("statH", statH_d, [128, KT, 3, 128], BF16)
        wkW = wload("wkW", wkW_d, [128, KT, 2, 128], BF16)
        wkH = wload("wkH", wkH_d, [128, KT, 2, 128], BF16)
        f2 = wload("f2", f2_d, [128, KT, 2, 128], BF16)
        fg = wload("fg", fg_d, [128, KT, 2, 128], BF16)

        bias_sb = wp.tile([128, 5, 2], F32, tag="biases")
        nc.scalar.dma_start(bias_sb[:], bias_d[:].transpose([2, 0, 1]))
        zb = wp.tile([128, 1], F32, tag="zb")
        nc.vector.memset(zb[:], 0.0)

        def bap(i, ct):
            return bias_sb[:, i, ct].unsqueeze(1)

        biasW = {
            "bv2": None if bvW0 else [bap(0, ct) for ct in range(2)],
            "bk2": None if bkW0 else [bap(1, ct) for ct in range(2)],
            "zb": zb[:],
        }
        biasH = {
            "bv2": None if bvH0 else [bap(2, ct) for ct in range(2)],
            "bk2": None if bkH0 else [bap(3, ct) for ct in range(2)],
            "zb": zb[:],
        }

        def load_x(b):
            x = xp.tile([128, KT, HW], BF16, tag="x")
            for kt in range(KT):
                for j in range(NCH):
                    nc.sync.dma_start(x[:, kt, bass.ts(j, CH)],
                                      x_d[b, bass.ts(kt, 128), bass.ts(j, CH)])
            return x

        def make_O(b):
            O_t = op_.tile([128, KT, HW], BF16, tag="O")
            return O_t

        # stage-W p3 chunk: O = x + gate*ctxb  (no matmul; feeds all H work)
        def w_p3_chunk(st, x, O, j):
            g2 = st.g2_chunk(j)
            g2f = g2[:].rearrange("p c a r -> p c (a r)")
            eng = nc.gpsimd if O_ON_GPSIMD else nc.vector
            eng.tensor_tensor(O[:, :, bass.ts(j, CH)], x[:, :, bass.ts(j, CH)], g2f, op=ALU.add)

        # stage-H p3 chunk: y = F2@O + FG@g2H, evict
        def h_p3_chunk(st, O, b, j):
            g2 = st.g2_chunk(j)
            ps_f = pvf.tile([128, 2 * CH], F32, tag="vf")
            sl = bass.ts(j, CH)
            for mt in range(2):
                half = ps_f[:, bass.ts(mt, CH)]
                nc.tensor.matmul(half, f2[:, 0, mt, :], O[:, 0, sl], start=True, stop=False)
                nc.tensor.matmul(half, f2[:, 1, mt, :], O[:, 1, sl], start=False, stop=False)
                nc.tensor.matmul(half, fg[:, 0, mt, :], g2[:, 0].rearrange("p a r -> p (a r)"), start=False, stop=False)
                nc.tensor.matmul(half, fg[:, 1, mt, :], g2[:, 1].rearrange("p a r -> p (a r)"), start=False, stop=True)
            y_t = yp.tile([128, 2, CH], F32, tag="y")
            if byH0:
                nc.scalar.activation(y_t[:], ps_f[:].rearrange("p (c n) -> p c n", c=2), AF.Copy)
            else:
                for ct in range(2):
                    nc.scalar.activation(y_t[:, ct, :], ps_f[:, bass.ts(ct, CH)],
                                         AF.Identity, bias=bap(4, ct))
            nc.sync.dma_start(
                y_d[b].rearrange("(c p) n -> p c n", p=128)[:, :, sl], y_t[:])

        x0 = load_x(0)
        x1 = load_x(1)
        w0 = _Stage(nc, pools, True, x0[:], statW, wkW, biasW)
        w1 = _Stage(nc, pools, True, x1[:], statW, wkW, biasW)

        def run_p1(st):
            st.p1_alloc()
            for j in range(NCH):
                st.p1_chunk(j)

        run_p1(w0)
        run_p1(w1)
        w0.p2()

        O0 = make_O(0)
        h0 = _Stage(nc, pools, False, O0[:], statH, wkH, biasH)
        h0.p1_alloc()
        for j in range(NCH):
            w_p3_chunk(w0, x0, O0, j)
            h0.p1_chunk(j)

        w1.p2()
        O1 = make_O(1)
        h1 = _Stage(nc, pools, False, O1[:], statH, wkH, biasH)
        h1.p1_alloc()
        for j in range(NCH):
            w_p3_chunk(w1, x1, O1, j)
            h1.p1_chunk(j)

        h0.p2()
        for j in range(NCH):
            h_p3_chunk(h0, O0, 0, j)
        h1.p2()
        for j in range(NCH):
            h_p3_chunk(h1, O1, 1, j)

    nc.compile()
    return nc


def _to_stat(wq, wv):
    """[q replicated; v] -> lhsT layout [128, KT, 3, 128]."""
    stat = np.empty((128, KT, 3, 128), np.float32)
    for kt in range(KT):
        cs = slice(kt * 128, (kt + 1) * 128)
        stat[:, kt, 0, :] = wv[0:128, cs].T
        stat[:, kt, 1, :] = wv[128:256, cs].T
        stat[:, kt, 2, :] = np.repeat(wq[cs][:, None], 128, axis=1)
    return stat


def _to_lhsT(w):
    """[256out, 256in] -> [128, KT, 2, 128] (k-tile, m-tile)."""
    t = np.empty((128, KT, 2, 128), np.float32)
    for kt in range(KT):
        cs = slice(kt * 128, (kt + 1) * 128)
        t[:, kt, 0, :] = w[0:128, cs].T
        t[:, kt, 1, :] = w[128:256, cs].T
    return t


def kernel(x, qkvW_w, qkvW_b, qkvH_w, qkvH_b, fusW_w, fusW_b, fusH_w, fusH_b):
    global LAST_RESULTS
    x = np.asarray(x, np.float32)
    f64 = lambda a: np.asarray(a, np.float64)
    qkvW_w, qkvW_b = f64(qkvW_w), f64(qkvW_b)
    qkvH_w, qkvH_b = f64(qkvH_w), f64(qkvH_b)
    fusW_w, fusW_b = f64(fusW_w), f64(fusW_b)
    fusH_w, fusH_b = f64(fusH_w), f64(fusH_b)

    wqW, wkW_m, wvW = qkvW_w[0], qkvW_w[1:1 + C], qkvW_w[1 + C:]
    wqH, wkH_m, wvH = qkvH_w[0], qkvH_w[1:1 + C], qkvH_w[1 + C:]

    # collapse WfW into stage-H weights (x_w = WfW @ O + bfW)
    wqHp = wqH @ fusW_w
    wvHp = wvH @ fusW_w
    wkHp = wkH_m @ fusW_w
    F2 = fusH_w @ fusW_w

    statW = _to_stat(wqW.astype(np.float32), wvW.astype(np.float32))
    statH = _to_stat(wqHp.astype(np.float32), wvHp.astype(np.float32))
    wkWl = _to_lhsT(wkW_m.astype(np.float32))
    wkHl = _to_lhsT(wkHp.astype(np.float32))
    f2l = _to_lhsT(F2.astype(np.float32))
    fgl = _to_lhsT(fusH_w.astype(np.float32))

    # bias constants (all zero for the graded inputs; exact host algebra)
    bvW = qkvW_b[1 + C:]
    bkW = qkvW_b[1:1 + C]
    bvHp = wvH @ fusW_b + qkvH_b[1 + C:]
    bkHp = wkH_m @ fusW_b + qkvH_b[1:1 + C]   # sum_h softmax = 1 -> adds to ctx
    byH = fusH_w @ fusW_b + fusH_b
    # tanh-gate folding: ACT computes tanh(0.5*v + 0.5*bv); ctx is scaled by
    # 0.5 on device, so the ctx bias constant also carries the 0.5.
    biases = np.stack([
        0.5 * bvW.reshape(2, 128), 0.5 * bkW.reshape(2, 128),
        0.5 * bvHp.reshape(2, 128), 0.5 * bkHp.reshape(2, 128),
        byH.reshape(2, 128),
    ]).astype(np.float32)

    flags = (
        not bvW.any(), not bkW.any(), not bvHp.any(), not bkHp.any(), not byH.any(),
    )
    if flags not in _BUILD_CACHE:
        _BUILD_CACHE[flags] = _build(flags)
    nc = _BUILD_CACHE[flags]

    tobf = lambda a: np.ascontiguousarray(a.astype(NPBF))
    xbf = np.ascontiguousarray(x.reshape(B, C, HW).astype(NPBF))
    in_maps = []
    for core in range(NCORES):
        in_maps.append({
            "x": xbf[core * BPC: (core + 1) * BPC],
            "statW": tobf(statW), "statH": tobf(statH),
            "wkW": tobf(wkWl), "wkH": tobf(wkHl),
            "f2": tobf(f2l), "fg": tobf(fgl),
            "biases": biases,
        })

    res = run_bass_kernel_spmd(nc, in_maps, list(range(NCORES)))
    LAST_RESULTS = res
    y = np.concatenate([r["y"] for r in res.results], axis=0)
    return y.reshape(B, C, H, W)
